# revision 2
# baseline (speedup 1.0000x reference)
"""GCN+GIN graph encoder on 8 Trainium2 NeuronCores (Bass/Tile).

Math (reference):
  GCNConv:  h = relu(segsum_dst(norm_e * (x@W0)[src]) + b0),
            norm_e = dinv[src]*dinv[dst] over edges+self-loops,
            dinv = rsqrt(deg incl self-loop)
  GIN x2:   h = relu((h + segsum_dst(h[src])) @ Wg + bg)
  pool:     m = segment_mean(h, batch) -> relu(m@Wh1+bh1)@Wh2+bh2

Distribution: nodes (and their in-edges) sharded contiguously over 8 cores.
Per layer each core aggregates messages for its own dst nodes by gathering
rows of a replicated node-feature table (dma_gather), reducing edge tiles
with one-hot selection matrices on the TensorEngine, applying the layer
linear transform W-stationary in feat-major, then transposing back to
node-major.  Tables are re-replicated between layers with an AllGather;
pooled partial means are combined with an AllReduce and the small MLP head
is computed redundantly on every core.

This revision optimizes per-exec wall cost, which is dominated by (a) NEFF
size (static instruction count) and (b) staged input bytes:
  - every block of 128 dst nodes is padded to a UNIFORM tile count per
    stream (NT_lo/NT_hi), making all addressing affine in the block id, so
    each layer runs as a single 49-iteration tc.For_i hardware loop
    (static NEFF ~300 instructions instead of ~6600);
  - staged inputs are compacted: x*dinv as bf16, dst offsets as uint8
    (pad=255 never matches iota), per-edge norm vals as bf16, gather
    indices stored once [16, cols] and replicated to 128 partitions
    on-device; the iota matrix is generated on-device.

Aggregation identity per dst block b (128 dst nodes):
  aggT[f, d] = sum_e msg[e, f] * sel[e, d],  sel[e, d] = (doff[e] == d) * val[e]
computed as matmul(lhsT=msg_tile[128e, 128f], rhs=sel[128e, 128d]) accumulated
in PSUM over the block's edge tiles.  GCN folds dinv[src] into the table rows
(host-prescaled x) and dinv[dst] into val; GIN uses val=1 and a self-loop edge
supplies the "+h" term.  Pad edge slots carry doff=255 -> zero contribution.
"""
import sys
import os

sys.path.insert(0, '/opt/trn_rl_repo')

import numpy as np
import ml_dtypes

import concourse.bass as bass
import concourse.bacc as bacc
import concourse.mybir as mybir
import concourse.tile as tile
from concourse.bass_utils import run_bass_kernel_spmd
from concourse.bass import ds
from concourse.masks import make_identity

F32 = mybir.dt.float32
BF16 = mybir.dt.bfloat16
U8 = mybir.dt.uint8
I16 = mybir.dt.int16
I32 = mybir.dt.int32
P = 128
NCORES = 8
NQ = 4                      # SWDGE queues
CHUNK = 8                   # tiles per dma_gather (1024 rows, single_packet)


class Cfg:
    def __init__(self, N, E, G, F, NHID, NOUT, NPN):
        self.N = N            # real nodes
        self.E = E            # edges (no self loops)
        self.G = G            # graphs
        self.F = F            # feature/hidden width (128)
        self.NHID = NHID
        self.NOUT = NOUT
        self.NPN = NPN        # real nodes per core
        assert NPN * NCORES >= N > NPN * (NCORES - 1)
        self.NPC = ((NPN + P - 1) // P) * P   # padded nodes per core
        self.NBLK = self.NPC // P
        self.NPAD = self.NPC * NCORES
        self.NHALF = self.NPAD // 2
        assert self.NHALF < 32768
        assert G == 2 * P


FULL = Cfg(N=50000, E=800000, G=256, F=128, NHID=256, NOUT=128, NPN=6250)


# ---------------------------------------------------------------- host prep
def preprocess(cfg, x, edge_index, batch, W0, b0, Wg1, bg1, Wg2, bg2,
               Wh1, bh1, Wh2, bh2):
    N, G, F = cfg.N, cfg.G, cfg.F
    NPN, NPC, NBLK, NHALF = cfg.NPN, cfg.NPC, cfg.NBLK, cfg.NHALF

    src = np.asarray(edge_index[0], dtype=np.int64)
    dst = np.asarray(edge_index[1], dtype=np.int64)
    batch = np.asarray(batch, dtype=np.int64)
    loop = np.arange(N, dtype=np.int64)
    s_all = np.concatenate([src, loop])
    d_all = np.concatenate([dst, loop])

    deg = np.bincount(d_all, minlength=N).astype(np.float64)
    dinv = (1.0 / np.sqrt(np.maximum(deg, 1.0))).astype(np.float32)

    def tabidx(n):
        c = n // NPN
        return c * NPC + (n - c * NPN)

    sidx = tabidx(s_all).astype(np.int64)
    c_e = d_all // NPN
    loc = d_all - c_e * NPN
    b_e = loc // P
    off_e = loc % P
    gblk = c_e * NBLK + b_e                      # global dst block id
    st_e = (sidx >= NHALF).astype(np.int64)      # stream: 0=lo, 1=hi
    si_e = sidx - st_e * NHALF                   # idx within half-table
    val_e = dinv[d_all].astype(np.float32)       # GCN dst scaling

    NGB = NCORES * NBLK
    # sort by (block, stream), rank within each group
    key = gblk * 2 + st_e
    order = np.argsort(key, kind="stable")
    kg, si, sof, sva = key[order], si_e[order], off_e[order], val_e[order]
    cnt = np.bincount(kg, minlength=NGB * 2)
    starts = np.zeros(NGB * 2, dtype=np.int64)
    starts[1:] = np.cumsum(cnt)[:-1]
    rank = np.arange(len(kg)) - np.repeat(starts, cnt)

    cnt2 = cnt.reshape(NCORES, NBLK, 2)
    NT_lo = int(np.ceil(cnt2[:, :, 0].max() / P))
    NT_hi = int(np.ceil(cnt2[:, :, 1].max() / P))
    NTT = NT_lo + NT_hi
    C = NBLK * NTT                               # doff/val columns per core

    c_of = kg // (2 * NBLK)
    b_of = (kg // 2) % NBLK
    st_of = kg % 2
    NT_st = np.where(st_of == 0, NT_lo, NT_hi)

    # idx arrays: per-stream per-core flat row position
    rows_lo, rows_hi = NBLK * NT_lo * P, NBLK * NT_hi * P
    pos_idx = b_of * (NT_st * P) + rank          # within (core, stream)
    idx_lo = np.zeros((NCORES, rows_lo), dtype=np.int16)
    idx_hi = np.zeros((NCORES, rows_hi), dtype=np.int16)
    m = st_of == 0
    idx_lo[c_of[m], pos_idx[m]] = si[m]
    m = st_of == 1
    idx_hi[c_of[m], pos_idx[m]] = si[m]
    # wrap for dma_gather: element e -> partition e%16, col e//16
    idx_lo = idx_lo.reshape(NCORES, rows_lo // 16, 16).transpose(0, 2, 1).copy()
    idx_hi = idx_hi.reshape(NCORES, rows_hi // 16, 16).transpose(0, 2, 1).copy()

    # doff/val: [128, C] col-major tiles; block b cols [b*NTT, (b+1)*NTT),
    # lo tiles first then hi.  Pad doff=255 (never equals iota 0..127).
    col_of = b_of * NTT + st_of * NT_lo + rank // P
    row_of = rank % P
    doff = np.full((NCORES, P, C), 255, dtype=np.uint8)
    val = np.zeros((NCORES, P, C), dtype=np.float32)
    doff[c_of, row_of, col_of] = sof
    val[c_of, row_of, col_of] = sva
    val = val.astype(ml_dtypes.bfloat16)

    # per-core node-feature slice, pre-scaled by dinv (GCN source scaling)
    xs = np.zeros((NCORES, NPC, F), dtype=np.float32)
    x = np.asarray(x, dtype=np.float32)
    for c in range(NCORES):
        lo_n = c * NPN
        hi_n = min(N, (c + 1) * NPN)
        n = hi_n - lo_n
        xs[c, :n] = x[lo_n:hi_n] * dinv[lo_n:hi_n, None]
    xs = xs.astype(ml_dtypes.bfloat16)

    # pooling metadata
    cnt_g = np.bincount(batch, minlength=G).astype(np.float32)
    invc = (1.0 / np.maximum(cnt_g, 1.0)).astype(np.float32)
    batA = np.full((NCORES, P, NBLK), -1.0, dtype=np.float32)
    batB = np.full((NCORES, P, NBLK), -1000.0, dtype=np.float32)
    for c in range(NCORES):
        lo_n = c * NPN
        hi_n = min(N, (c + 1) * NPN)
        n = hi_n - lo_n
        bb = batch[lo_n:hi_n].astype(np.float32)
        colmaj = np.full(NPC, -1.0, dtype=np.float32)
        colmaj[:n] = bb
        batA[c] = colmaj.reshape(NBLK, P).T
        batB[c] = batA[c] - 128.0
        batA[c][batA[c] < 0] = -1.0

    common = dict(
        w0=np.asarray(W0, np.float32), wg1=np.asarray(Wg1, np.float32),
        wg2=np.asarray(Wg2, np.float32),
        b0c=np.asarray(b0, np.float32).reshape(P, 1).copy(),
        bg1c=np.asarray(bg1, np.float32).reshape(P, 1).copy(),
        bg2c=np.asarray(bg2, np.float32).reshape(P, 1).copy(),
        wh1=np.asarray(Wh1, np.float32),
        bh1c=np.asarray(bh1, np.float32).reshape(2, P).T.copy(),  # [128,2]
        wh2=np.asarray(Wh2, np.float32),
        bh2rep=np.broadcast_to(np.asarray(bh2, np.float32), (P, cfg.NOUT)).copy(),
        invcA=invc[:P].reshape(P, 1).copy(),
        invcB=invc[P:].reshape(P, 1).copy(),
    )
    in_maps = []
    for c in range(NCORES):
        mdict = dict(common)
        mdict.update(
            xs=xs[c],
            idxlo=idx_lo[c], idxhi=idx_hi[c],
            doff=doff[c], val=val[c],
            batA=batA[c], batB=batB[c],
        )
        in_maps.append(mdict)
    meta = dict(NT_lo=NT_lo, NT_hi=NT_hi)
    return in_maps, meta


# ---------------------------------------------------------------- program
def build_program(cfg, meta):
    NPC, NBLK, NPAD, NHALF = cfg.NPC, cfg.NBLK, cfg.NPAD, cfg.NHALF
    F, NHID, NOUT, G = cfg.F, cfg.NHID, cfg.NOUT, cfg.G
    NT_lo, NT_hi = meta["NT_lo"], meta["NT_hi"]
    NTT = NT_lo + NT_hi
    C = NBLK * NTT
    rows_lo, rows_hi = NBLK * NT_lo * P, NBLK * NT_hi * P

    nc = bacc.Bacc(None, target_bir_lowering=False, debug=True,
                   num_devices=NCORES, num_swdge_queues=NQ)

    def din(name, shape, dt=F32):
        return nc.declare_dram_parameter(name, list(shape), dt, isOutput=False)

    xs_d = din("xs", [NPC, F], BF16)
    idxlo_d = din("idxlo", [16, rows_lo // 16], I16)
    idxhi_d = din("idxhi", [16, rows_hi // 16], I16)
    doff_d = din("doff", [P, C], U8)
    val_d = din("val", [P, C], BF16)
    w0_d = din("w0", [F, F]); wg1_d = din("wg1", [F, F]); wg2_d = din("wg2", [F, F])
    b0c_d = din("b0c", [P, 1]); bg1c_d = din("bg1c", [P, 1]); bg2c_d = din("bg2c", [P, 1])
    wh1_d = din("wh1", [F, NHID]); bh1c_d = din("bh1c", [P, 2])
    wh2_d = din("wh2", [NHID, NOUT]); bh2rep_d = din("bh2rep", [P, NOUT])
    batA_d = din("batA", [P, NBLK]); batB_d = din("batB", [P, NBLK])
    invcA_d = din("invcA", [P, 1]); invcB_d = din("invcB", [P, 1])
    out_d = nc.declare_dram_parameter("out", [G, NOUT], F32, isOutput=True)

    slice0 = nc.dram_tensor("slice0", [NPC, F], F32)
    slice1 = nc.dram_tensor("slice1", [NPC, F], F32)
    slice2 = nc.dram_tensor("slice2", [NPC, F], F32)
    tab1 = nc.dram_tensor("tab1", [NPAD, F], F32)
    tab2 = nc.dram_tensor("tab2", [NPAD, F], F32)
    tab3 = nc.dram_tensor("tab3", [NPAD, F], F32)
    pool_in = nc.dram_tensor("pool_in", [G, F], F32)
    pool_out = nc.dram_tensor("pool_out", [G, F], F32, addr_space="Shared")
    groups = [list(range(NCORES))]

    # static chunking of a stream's tiles into dma_gathers of <=CHUNK tiles
    def chunks(NT):
        out = []
        t0 = 0
        while t0 < NT:
            nt = min(CHUNK, NT - t0)
            out.append((t0, nt))
            t0 += nt
        return out

    with tile.TileContext(nc) as tc:
        with (
            tc.tile_pool(name="const", bufs=1) as constp,
            tc.tile_pool(name="gbuf", bufs=1) as gbufp,
            tc.tile_pool(name="sel", bufs=4) as selp,
            tc.tile_pool(name="work", bufs=4) as workp,
            tc.tile_pool(name="pagg", bufs=2, space="PSUM") as pagg,
            tc.tile_pool(name="phT", bufs=2, space="PSUM") as phT,
            tc.tile_pool(name="ptr", bufs=1, space="PSUM") as ptr,
            tc.tile_pool(name="ppool", bufs=1, space="PSUM") as ppool,
        ):
            # ---- constants / metadata to SBUF
            ident = constp.tile([P, P], F32)
            make_identity(nc, ident[:])
            iota_i = constp.tile([P, P], I32)
            nc.gpsimd.iota(out=iota_i[:], pattern=[[1, P]], base=0,
                           channel_multiplier=0)
            iota = constp.tile([P, P], F32)
            nc.vector.tensor_copy(out=iota[:], in_=iota_i[:])
            zero_sb = constp.tile([P, P], F32)
            nc.vector.memset(zero_sb[:], 0.0)

            def load(t_shape, dram, dt=F32, pool=constp):
                nm = f"sb_{dram.name}"
                t = pool.tile(list(t_shape), dt, name=nm, tag=nm)
                nc.sync.dma_start(out=t[:], in_=dram[:])
                return t

            # gather idx: load compact [16, cols], replicate to 128 partitions
            def load_idx(dram, cols):
                t16 = constp.tile([16, cols], I16, tag=f"t16_{dram.name}")
                nc.sync.dma_start(out=t16[:], in_=dram[:])
                t128 = constp.tile([P, cols], I16, tag=f"t128_{dram.name}")
                for k in range(8):
                    nc.sync.dma_start(out=t128[k * 16:(k + 1) * 16, :], in_=t16[:])
                return t128

            idxlo = load_idx(idxlo_d, rows_lo // 16)
            idxhi = load_idx(idxhi_d, rows_hi // 16)
            doff_u = load([P, C], doff_d, U8)
            doff = constp.tile([P, C], F32)
            nc.vector.tensor_copy(out=doff[:], in_=doff_u[:])
            val_b = load([P, C], val_d, BF16)
            val = constp.tile([P, C], F32)
            nc.vector.tensor_copy(out=val[:], in_=val_b[:])

            w0 = load([F, F], w0_d)
            wg1 = load([F, F], wg1_d)
            wg2 = load([F, F], wg2_d)
            b0c = load([P, 1], b0c_d)
            bg1c = load([P, 1], bg1c_d)
            bg2c = load([P, 1], bg2c_d)
            wh1 = load([F, NHID], wh1_d)
            bh1c = load([P, 2], bh1c_d)
            wh2 = constp.tile([P, (NHID // P) * NOUT], F32)
            for h in range(NHID // P):
                nc.sync.dma_start(out=wh2[:, h * NOUT:(h + 1) * NOUT],
                                  in_=wh2_d[h * P:(h + 1) * P, :])
            bh2rep = load([P, NOUT], bh2rep_d)
            batA = load([P, NBLK], batA_d)
            batB = load([P, NBLK], batB_d)
            invcA = load([P, 1], invcA_d)
            invcB = load([P, 1], invcB_d)

            # ---- xs bf16 -> slice0 f32 (collectives need internal tensors)
            with tc.For_i(0, NBLK, 1, name="cvt") as i:
                xb = workp.tile([P, F], BF16, tag="xb")
                nc.sync.dma_start(out=xb[:], in_=xs_d[ds(i * P, P), :])
                xf = workp.tile([P, F], F32, tag="xf")
                nc.vector.tensor_copy(out=xf[:], in_=xb[:])
                nc.sync.dma_start(out=slice0[ds(i * P, P), :], in_=xf[:])
            nc.gpsimd.collective_compute(
                "AllGather", mybir.AluOpType.bypass, replica_groups=groups,
                ins=[slice0[:]], outs=[tab1[:]])

            # persistent gather buffers (shared across the 3 layers)
            buf_lo = gbufp.tile([P, NT_lo * P], F32, tag="buflo")
            buf_hi = gbufp.tile([P, NT_hi * P], F32, tag="bufhi")
            # pooling accumulators, pre-zeroed; in-loop matmuls use
            # start=False and a dummy zero matmul closes the group.
            pool_ps = {}
            for half in ("A", "B"):
                t = ppool.tile([P, F], F32, space="PSUM",
                               tag=f"pool{half}", name=f"pool{half}")
                nc.vector.memset(t[:], 0.0)
                pool_ps[half] = t

            def emit_layer(L, tab, W_sb, bias_col, use_val, out_slice):
                qi = 0
                with tc.For_i(0, NBLK, 1, name=f"layer{L}") as i:
                    streams = [
                        ("lo", NT_lo, idxlo, buf_lo, tab[0:NHALF, :], 0),
                        ("hi", NT_hi, idxhi, buf_hi, tab[NHALF:NPAD, :], NT_lo),
                    ]
                    for sname, NT, idx_sb, buf, tab_ap, coff in streams:
                        for (t0, nt) in chunks(NT):
                            nc.gpsimd.dma_gather(
                                out_ap=buf[:, t0 * P:(t0 + nt) * P].rearrange(
                                    "p (c f) -> p c f", f=F),
                                in_ap=tab_ap,
                                idxs_ap=idx_sb[:, ds(i * NT * 8 + t0 * 8, nt * 8)],
                                num_idxs=nt * P, num_idxs_reg=nt * P,
                                elem_size=F, single_packet=True,
                                queue_num=qi % NQ)
                            qi += 1
                    agg_ps = pagg.tile([P, F], F32, space="PSUM", tag="agg")
                    k = 0
                    for sname, NT, idx_sb, buf, tab_ap, coff in streams:
                        for t in range(NT):
                            sel = selp.tile([P, P], F32)
                            col = ds(i * NTT + coff + t, 1)
                            if use_val:
                                nc.vector.tensor_scalar(
                                    out=sel[:], in0=iota[:],
                                    scalar1=doff[:, col], scalar2=val[:, col],
                                    op0=mybir.AluOpType.is_equal,
                                    op1=mybir.AluOpType.mult)
                            else:
                                nc.vector.tensor_scalar(
                                    out=sel[:], in0=iota[:],
                                    scalar1=doff[:, col], scalar2=None,
                                    op0=mybir.AluOpType.is_equal)
                            nc.tensor.matmul(
                                out=agg_ps[:], lhsT=buf[:, t * P:(t + 1) * P],
                                rhs=sel[:], start=(k == 0), stop=(k == NTT - 1))
                            k += 1
                    aggT = workp.tile([P, F], F32, tag="aggT")
                    nc.vector.tensor_copy(out=aggT[:], in_=agg_ps[:])
                    hT_ps = phT.tile([P, F], F32, space="PSUM", tag="hT")
                    nc.tensor.matmul(out=hT_ps[:], lhsT=W_sb[:], rhs=aggT[:],
                                     start=True, stop=True)
                    hT = workp.tile([P, F], F32, tag="hT_sb")
                    nc.scalar.activation(out=hT[:], in_=hT_ps[:],
                                         func=mybir.ActivationFunctionType.Relu,
                                         bias=bias_col[:, 0:1])
                    h_ps = ptr.tile([P, F], F32, space="PSUM", tag="tr")
                    nc.tensor.transpose(out=h_ps[:], in_=hT[:], identity=ident[:])
                    h_sb = workp.tile([P, F], F32, tag="h_sb")
                    nc.vector.tensor_copy(out=h_sb[:], in_=h_ps[:])
                    if out_slice is not None:
                        nc.sync.dma_start(out=out_slice[ds(i * P, P), :],
                                          in_=h_sb[:])
                    else:
                        for half, bat in (("A", batA), ("B", batB)):
                            selp_t = selp.tile([P, P], F32)
                            nc.vector.tensor_scalar(
                                out=selp_t[:], in0=iota[:],
                                scalar1=bat[:, ds(i, 1)], scalar2=None,
                                op0=mybir.AluOpType.is_equal)
                            nc.tensor.matmul(
                                out=pool_ps[half][:], lhsT=selp_t[:],
                                rhs=h_sb[:], start=False, stop=False)

            emit_layer(0, tab1, w0, b0c, True, slice1)
            nc.gpsimd.collective_compute(
                "AllGather", mybir.AluOpType.bypass, replica_groups=groups,
                ins=[slice1[:]], outs=[tab2[:]])
            emit_layer(1, tab2, wg1, bg1c, False, slice2)
            nc.gpsimd.collective_compute(
                "AllGather", mybir.AluOpType.bypass, replica_groups=groups,
                ins=[slice2[:]], outs=[tab3[:]])
            emit_layer(2, tab3, wg2, bg2c, False, None)

            # ---- pooling: close PSUM groups, partial means -> AllReduce
            for half, invc in (("A", invcA), ("B", invcB)):
                nc.tensor.matmul(out=pool_ps[half][:], lhsT=zero_sb[:],
                                 rhs=zero_sb[:], start=False, stop=True)
                m_sb = workp.tile([P, F], F32, tag=f"m{half}")
                nc.vector.tensor_scalar(
                    out=m_sb[:], in0=pool_ps[half][:], scalar1=invc[:, 0:1],
                    scalar2=None, op0=mybir.AluOpType.mult)
                base = 0 if half == "A" else P
                nc.sync.dma_start(out=pool_in[base:base + P, :], in_=m_sb[:])
            nc.gpsimd.collective_compute(
                "AllReduce", mybir.AluOpType.add, replica_groups=groups,
                ins=[pool_in[:]], outs=[pool_out[:]])

            # ---- head (redundant on every core)
            g1T = {}
            for hi, half in enumerate(("A", "B")):
                m_sb = workp.tile([P, F], F32, tag=f"mf{half}")
                nc.sync.dma_start(out=m_sb[:], in_=pool_out[hi * P:(hi + 1) * P, :])
                mT_ps = phT.tile([P, F], F32, space="PSUM", tag="hT")
                nc.tensor.transpose(out=mT_ps[:], in_=m_sb[:], identity=ident[:])
                mT = workp.tile([P, F], F32, tag=f"mT{half}")
                nc.vector.tensor_copy(out=mT[:], in_=mT_ps[:])
                for h in range(NHID // P):
                    g_ps = pagg.tile([P, P], F32, space="PSUM", tag="agg")
                    nc.tensor.matmul(out=g_ps[:], lhsT=wh1[:, h * P:(h + 1) * P],
                                     rhs=mT[:], start=True, stop=True)
                    gt = workp.tile([P, P], F32, tag=f"g1T{half}{h}")
                    nc.scalar.activation(out=gt[:], in_=g_ps[:],
                                         func=mybir.ActivationFunctionType.Relu,
                                         bias=bh1c[:, h:h + 1])
                    g1T[(half, h)] = gt
            for hi, half in enumerate(("A", "B")):
                o_ps = pagg.tile([P, NOUT], F32, space="PSUM", tag="agg")
                for h in range(NHID // P):
                    nc.tensor.matmul(out=o_ps[:], lhsT=g1T[(half, h)][:],
                                     rhs=wh2[:, h * NOUT:(h + 1) * NOUT],
                                     start=(h == 0), stop=(h == NHID // P - 1))
                o_sb = workp.tile([P, NOUT], F32, tag=f"o{half}")
                nc.vector.tensor_add(out=o_sb[:], in0=o_ps[:], in1=bh2rep[:])
                nc.sync.dma_start(out=out_d[hi * P:(hi + 1) * P, :], in_=o_sb[:])

    nc.compile()
    return nc


_CACHE = {}


def run(cfg, inputs):
    in_maps, meta = preprocess(cfg, **inputs)
    key = (cfg.N, meta["NT_lo"], meta["NT_hi"])
    if key not in _CACHE:
        _CACHE[key] = build_program(cfg, meta)
    nc = _CACHE[key]
    res = run_bass_kernel_spmd(nc, in_maps, core_ids=list(range(NCORES)))
    return res.results[0]["out"].astype(np.float32)


def kernel(**inputs):
    return run(FULL, inputs)


# revision 6
# speedup vs baseline: 1.3411x; 1.3411x over previous
"""GCN+GIN graph encoder on 8 Trainium2 NeuronCores (Bass/Tile).

Math (reference):
  GCNConv:  h = relu(segsum_dst(norm_e * (x@W0)[src]) + b0),
            norm_e = dinv[src]*dinv[dst] over edges+self-loops,
            dinv = rsqrt(deg incl self-loop)
  GIN x2:   h = relu((h + segsum_dst(h[src])) @ Wg + bg)
  pool:     m = segment_mean(h, batch) -> relu(m@Wh1+bh1)@Wh2+bh2

Distribution: nodes (and their in-edges) sharded contiguously over 8 cores.
Per layer each core aggregates messages for its own dst nodes by gathering
rows of a replicated node-feature table (dma_gather), reducing edge tiles
with one-hot selection matrices on the TensorEngine, applying the layer
linear transform W-stationary, then writing its node slice back; tables are
re-replicated between layers with an AllGather.  Pooled partial means are
combined with an AllReduce and the small MLP head runs redundantly per core.

Per-exec wall cost here is dominated by (a) NEFF size (static instruction
count), (b) staged input bytes, and (c) a ~5ms per-parameter overhead, so:
  - every block of 128 dst nodes is padded to a UNIFORM tile count per
    stream (NT_lo/NT_hi), making all addressing affine in the block id, so
    each layer runs as one 49-iteration tc.For_i hardware loop (static NEFF
    ~300 instructions instead of ~6600);
  - ALL inputs ship as ONE bf16 [128, X] parameter `pk`: x*dinv blocks,
    bf16 weights/biases/batch-ids, uint8 dst-offsets and int16 gather
    indices ride along via bitcast views (reshape-DMA re-wraps the indices
    to 16 partitions on device), and the f32 1/count pooling scales ride as
    Dekker hi+lo bf16 pairs;
  - GCN's dinv[dst] factor is applied as a per-partition post-scale on a
    node-major aggregate (lhsT=sel matmul orientation) instead of per-edge
    `val` entries, eliminating that array entirely.

Aggregation identity per dst block b (128 dst nodes):
  layer 0:  agg[d, f] = sum_e sel[e, d] * msg[e, f]   (node-major, then
            scaled by dinv[d] per partition)
  layers 1+: aggT[f, d] = sum_e msg[e, f] * sel[e, d] (feat-major)
with sel[e, d] = (doff[e] == d), computed on the TensorEngine accumulated
in PSUM over the block's edge tiles.  GCN folds dinv[src] into the table
rows (host-prescaled x); GIN self-loop edges supply the "+h" term.  Pad
edge slots carry doff=255 -> zero contribution.
"""
import sys
import os

sys.path.insert(0, '/opt/trn_rl_repo')

import numpy as np
import ml_dtypes

import concourse.bass as bass
import concourse.bacc as bacc
import concourse.mybir as mybir
import concourse.tile as tile
from concourse.bass_utils import run_bass_kernel_spmd
from concourse.bass import ds
from concourse.masks import make_identity

F32 = mybir.dt.float32
BF16 = mybir.dt.bfloat16
U8 = mybir.dt.uint8
I16 = mybir.dt.int16
I32 = mybir.dt.int32
P = 128
NCORES = 8
NQ = 4                      # SWDGE queues
CHUNK = 8                   # tiles per dma_gather (1024 rows, single_packet)


class Cfg:
    def __init__(self, N, E, G, F, NHID, NOUT, NPN):
        self.N = N            # real nodes
        self.E = E            # edges (no self loops)
        self.G = G            # graphs
        self.F = F            # feature/hidden width (128)
        self.NHID = NHID
        self.NOUT = NOUT
        self.NPN = NPN        # real nodes per core
        assert NPN * NCORES >= N > NPN * (NCORES - 1)
        self.NPC = ((NPN + P - 1) // P) * P   # padded nodes per core
        self.NBLK = self.NPC // P
        self.NPAD = self.NPC * NCORES
        self.NHALF = self.NPAD // 2
        assert self.NHALF < 32768
        assert G == 2 * P


FULL = Cfg(N=50000, E=800000, G=256, F=128, NHID=256, NOUT=128, NPN=6250)


def pk_layout(cfg, NT_lo, NT_hi):
    """Column offsets into the single packed bf16 [128, X] parameter."""
    NBLK, F, NHID, NOUT = cfg.NBLK, cfg.F, cfg.NHID, cfg.NOUT
    NTT = NT_lo + NT_hi
    C = NBLK * NTT
    rows_lo, rows_hi = NBLK * NT_lo * P, NBLK * NT_hi * P
    o, L = 0, {}

    def put(name, w):
        nonlocal o
        L[name] = o
        o += w

    put("xs", NBLK * F)
    put("w0", F); put("wg1", F); put("wg2", F)
    put("wh1", NHID); put("wh2", (NHID // P) * NOUT)
    put("b0c", 1); put("bg1c", 1); put("bg2c", 1)
    put("bh1c", NHID // P); put("bh2col", 1)
    put("bat", 2 * NBLK)          # batA cols then batB cols
    put("dinvb", NBLK)
    put("invc", 4)                # hi A, hi B, lo A, lo B
    assert C % 2 == 0
    put("doff", C // 2)           # u8 [128, C] as bf16 cols
    assert rows_lo % P == 0 and rows_hi % P == 0
    put("idxlo", rows_lo // P)    # i16 flat as bf16 cols
    put("idxhi", rows_hi // P)
    L["_total"] = o
    return L


# ---------------------------------------------------------------- host prep
def preprocess(cfg, x, edge_index, batch, W0, b0, Wg1, bg1, Wg2, bg2,
               Wh1, bh1, Wh2, bh2):
    N, G, F = cfg.N, cfg.G, cfg.F
    NPN, NPC, NBLK, NHALF = cfg.NPN, cfg.NPC, cfg.NBLK, cfg.NHALF
    NHID, NOUT = cfg.NHID, cfg.NOUT

    src = np.asarray(edge_index[0], dtype=np.int64)
    dst = np.asarray(edge_index[1], dtype=np.int64)
    batch = np.asarray(batch, dtype=np.int64)
    loop = np.arange(N, dtype=np.int64)
    s_all = np.concatenate([src, loop])
    d_all = np.concatenate([dst, loop])

    deg = np.bincount(d_all, minlength=N).astype(np.float64)
    dinv = (1.0 / np.sqrt(np.maximum(deg, 1.0))).astype(np.float32)

    def tabidx(n):
        c = n // NPN
        return c * NPC + (n - c * NPN)

    sidx = tabidx(s_all).astype(np.int64)
    c_e = d_all // NPN
    loc = d_all - c_e * NPN
    b_e = loc // P
    off_e = loc % P
    gblk = c_e * NBLK + b_e                      # global dst block id
    st_e = (sidx >= NHALF).astype(np.int64)      # stream: 0=lo, 1=hi
    si_e = sidx - st_e * NHALF                   # idx within half-table

    NGB = NCORES * NBLK
    key = gblk * 2 + st_e
    order = np.argsort(key, kind="stable")
    kg, si, sof = key[order], si_e[order], off_e[order]
    cnt = np.bincount(kg, minlength=NGB * 2)
    starts = np.zeros(NGB * 2, dtype=np.int64)
    starts[1:] = np.cumsum(cnt)[:-1]
    rank = np.arange(len(kg)) - np.repeat(starts, cnt)

    cnt2 = cnt.reshape(NCORES, NBLK, 2)
    NT_lo = int(np.ceil(cnt2[:, :, 0].max() / P))
    NT_hi = int(np.ceil(cnt2[:, :, 1].max() / P))
    NTT = NT_lo + NT_hi
    C = NBLK * NTT
    rows_lo, rows_hi = NBLK * NT_lo * P, NBLK * NT_hi * P
    L = pk_layout(cfg, NT_lo, NT_hi)
    bf = ml_dtypes.bfloat16

    c_of = kg // (2 * NBLK)
    b_of = (kg // 2) % NBLK
    st_of = kg % 2
    NT_st = np.where(st_of == 0, NT_lo, NT_hi)

    pos_idx = b_of * (NT_st * P) + rank          # within (core, stream)
    idx_lo = np.zeros((NCORES, rows_lo), dtype=np.int16)
    idx_hi = np.zeros((NCORES, rows_hi), dtype=np.int16)
    m = st_of == 0
    idx_lo[c_of[m], pos_idx[m]] = si[m]
    m = st_of == 1
    idx_hi[c_of[m], pos_idx[m]] = si[m]
    # wrap for dma_gather (element e -> partition e%16, col e//16), then
    # flatten partition-major for the packed bf16 view
    idx_lo = idx_lo.reshape(NCORES, rows_lo // 16, 16).transpose(0, 2, 1)
    idx_hi = idx_hi.reshape(NCORES, rows_hi // 16, 16).transpose(0, 2, 1)
    idx_lo = np.ascontiguousarray(idx_lo).reshape(NCORES, -1).view(bf).reshape(
        NCORES, P, rows_lo // P)
    idx_hi = np.ascontiguousarray(idx_hi).reshape(NCORES, -1).view(bf).reshape(
        NCORES, P, rows_hi // P)

    col_of = b_of * NTT + st_of * NT_lo + rank // P
    row_of = rank % P
    doff = np.full((NCORES, P, C), 255, dtype=np.uint8)
    doff[c_of, row_of, col_of] = sof

    # per-core pk assembly
    x = np.asarray(x, dtype=np.float32)
    cnt_g = np.bincount(batch, minlength=G).astype(np.float32)
    invc = (1.0 / np.maximum(cnt_g, 1.0)).astype(np.float32)
    invc_hi = invc.astype(bf)
    invc_lo = (invc - invc_hi.astype(np.float32)).astype(bf)

    in_maps = []
    for c in range(NCORES):
        pk = np.zeros((P, L["_total"]), dtype=bf)
        lo_n = c * NPN
        hi_n = min(N, (c + 1) * NPN)
        n = hi_n - lo_n
        xsl = np.zeros((NPC, F), dtype=np.float32)
        xsl[:n] = x[lo_n:hi_n] * dinv[lo_n:hi_n, None]
        # xs block-major: col b*F+f = xs[b*P+p, f]
        pk[:, L["xs"]:L["xs"] + NBLK * F] = (
            xsl.reshape(NBLK, P, F).transpose(1, 0, 2).reshape(P, NBLK * F)
        ).astype(bf)
        pk[:, L["w0"]:L["w0"] + F] = np.asarray(W0, np.float32).astype(bf)
        pk[:, L["wg1"]:L["wg1"] + F] = np.asarray(Wg1, np.float32).astype(bf)
        pk[:, L["wg2"]:L["wg2"] + F] = np.asarray(Wg2, np.float32).astype(bf)
        pk[:, L["wh1"]:L["wh1"] + NHID] = np.asarray(Wh1, np.float32).astype(bf)
        wh2 = np.asarray(Wh2, np.float32)
        for h in range(NHID // P):
            pk[:, L["wh2"] + h * NOUT:L["wh2"] + (h + 1) * NOUT] = (
                wh2[h * P:(h + 1) * P, :].astype(bf))
        pk[:, L["b0c"]] = np.asarray(b0, np.float32).astype(bf)
        pk[:, L["bg1c"]] = np.asarray(bg1, np.float32).astype(bf)
        pk[:, L["bg2c"]] = np.asarray(bg2, np.float32).astype(bf)
        bh1m = np.asarray(bh1, np.float32).reshape(NHID // P, P).T
        pk[:, L["bh1c"]:L["bh1c"] + NHID // P] = bh1m.astype(bf)
        pk[:, L["bh2col"]] = np.asarray(bh2, np.float32).astype(bf)
        # batch ids: batA = id or -1 pad; batB = id-128 (pad -> -1128)
        colmaj = np.full(NPC, -1.0, dtype=np.float32)
        colmaj[:n] = batch[lo_n:hi_n].astype(np.float32)
        batA = colmaj.reshape(NBLK, P).T
        batB = batA - 128.0
        batA = batA.copy()
        batA[batA < 0] = -1.0
        pk[:, L["bat"]:L["bat"] + NBLK] = batA.astype(bf)
        pk[:, L["bat"] + NBLK:L["bat"] + 2 * NBLK] = batB.astype(bf)
        dvb = np.zeros(NPC, dtype=np.float32)
        dvb[:n] = dinv[lo_n:hi_n]
        pk[:, L["dinvb"]:L["dinvb"] + NBLK] = dvb.reshape(NBLK, P).T.astype(bf)
        pk[:, L["invc"] + 0] = invc_hi[:P]
        pk[:, L["invc"] + 1] = invc_hi[P:]
        pk[:, L["invc"] + 2] = invc_lo[:P]
        pk[:, L["invc"] + 3] = invc_lo[P:]
        pk[:, L["doff"]:L["doff"] + C // 2] = doff[c].view(bf)
        pk[:, L["idxlo"]:L["idxlo"] + rows_lo // P] = idx_lo[c]
        pk[:, L["idxhi"]:L["idxhi"] + rows_hi // P] = idx_hi[c]
        in_maps.append({"pk": pk})
    meta = dict(NT_lo=NT_lo, NT_hi=NT_hi)
    return in_maps, meta


# ---------------------------------------------------------------- program
def build_program(cfg, meta):
    NPC, NBLK, NPAD, NHALF = cfg.NPC, cfg.NBLK, cfg.NPAD, cfg.NHALF
    F, NHID, NOUT, G = cfg.F, cfg.NHID, cfg.NOUT, cfg.G
    NT_lo, NT_hi = meta["NT_lo"], meta["NT_hi"]
    NTT = NT_lo + NT_hi
    C = NBLK * NTT
    rows_lo, rows_hi = NBLK * NT_lo * P, NBLK * NT_hi * P
    L = pk_layout(cfg, NT_lo, NT_hi)

    nc = bacc.Bacc(None, target_bir_lowering=False, debug=True,
                   num_devices=NCORES, num_swdge_queues=NQ)

    pk_d = nc.declare_dram_parameter("pk", [P, L["_total"]], BF16,
                                     isOutput=False)
    out_d = nc.declare_dram_parameter("out", [G, NOUT], F32, isOutput=True)

    slice0 = nc.dram_tensor("slice0", [NPC, F], F32)
    slice1 = nc.dram_tensor("slice1", [NPC, F], F32)
    slice2 = nc.dram_tensor("slice2", [NPC, F], F32)
    tab1 = nc.dram_tensor("tab1", [NPAD, F], F32)
    tab2 = nc.dram_tensor("tab2", [NPAD, F], F32)
    tab3 = nc.dram_tensor("tab3", [NPAD, F], F32)
    pool_in = nc.dram_tensor("pool_in", [G, F], F32)
    pool_out = nc.dram_tensor("pool_out", [G, F], F32, addr_space="Shared")
    groups = [list(range(NCORES))]

    def chunks(NT):
        out, t0 = [], 0
        while t0 < NT:
            nt = min(CHUNK, NT - t0)
            out.append((t0, nt))
            t0 += nt
        return out

    with tile.TileContext(nc) as tc:
        with (
            tc.tile_pool(name="const", bufs=1) as constp,
            tc.tile_pool(name="gbuf", bufs=1) as gbufp,
            tc.tile_pool(name="sel", bufs=4) as selp,
            tc.tile_pool(name="work", bufs=4) as workp,
            tc.tile_pool(name="pagg", bufs=2, space="PSUM") as pagg,
            tc.tile_pool(name="phT", bufs=1, space="PSUM") as phT,
            tc.tile_pool(name="ptr", bufs=1, space="PSUM") as ptr,
            tc.tile_pool(name="ppool", bufs=1, space="PSUM") as ppool,
        ):
            pk = constp.tile([P, L["_total"]], BF16)
            nc.sync.dma_start(out=pk[:], in_=pk_d[:])

            ident = constp.tile([P, P], F32)
            make_identity(nc, ident[:])
            iota_i = constp.tile([P, P], I32)
            nc.gpsimd.iota(out=iota_i[:], pattern=[[1, P]], base=0,
                           channel_multiplier=0)
            iota = constp.tile([P, P], F32)
            nc.vector.tensor_copy(out=iota[:], in_=iota_i[:])
            zero_sb = constp.tile([P, P], F32)
            nc.vector.memset(zero_sb[:], 0.0)
            ones1 = constp.tile([1, P], F32)
            nc.vector.memset(ones1[:], 1.0)

            def unpack(name, w, dt=F32):
                t = constp.tile([P, w], dt, tag=f"u_{name}")
                nc.vector.tensor_copy(out=t[:], in_=pk[:, L[name]:L[name] + w])
                return t

            # gather idx: 8 reshape-DMAs replicate the 16-partition wrap
            def load_idx(name, rows):
                cols16 = rows // 16
                t128 = constp.tile([P, cols16], I16, tag=f"t128_{name}")
                src_ap = pk[:, L[name]:L[name] + rows // P].bitcast(I16)
                for k in range(8):
                    nc.sync.dma_start(out=t128[k * 16:(k + 1) * 16, :],
                                      in_=src_ap)
                return t128

            idxlo = load_idx("idxlo", rows_lo)
            idxhi = load_idx("idxhi", rows_hi)
            doff = constp.tile([P, C], F32)
            nc.vector.tensor_copy(
                out=doff[:], in_=pk[:, L["doff"]:L["doff"] + C // 2].bitcast(U8))

            w0 = unpack("w0", F)
            wg1 = unpack("wg1", F)
            wg2 = unpack("wg2", F)
            wh1 = unpack("wh1", NHID)
            wh2 = unpack("wh2", (NHID // P) * NOUT)
            b0c = unpack("b0c", 1)
            bg1c = unpack("bg1c", 1)
            bg2c = unpack("bg2c", 1)
            bh1c = unpack("bh1c", NHID // P)
            batf = unpack("bat", 2 * NBLK)
            dinvb = unpack("dinvb", NBLK)
            invch = unpack("invc", 4)
            invcf = constp.tile([P, 2], F32, tag="invcf")
            nc.vector.tensor_add(out=invcf[:], in0=invch[:, 0:2],
                                 in1=invch[:, 2:4])
            # bh2 row: [128,1] col -> reshape-DMA -> [1,128] -> f32
            bh2r_b = constp.tile([1, P], BF16, tag="bh2rb")
            nc.sync.dma_start(out=bh2r_b[:],
                              in_=pk[:, L["bh2col"]:L["bh2col"] + 1])
            bh2row = constp.tile([1, P], F32, tag="bh2row")
            nc.vector.tensor_copy(out=bh2row[:], in_=bh2r_b[:])

            # ---- xs bf16 -> slice0 f32 (collectives need internal tensors)
            with tc.For_i(0, NBLK, 1, name="cvt") as i:
                xf = workp.tile([P, F], F32, tag="xf")
                nc.vector.tensor_copy(out=xf[:],
                                      in_=pk[:, ds(L["xs"] + i * F, F)])
                nc.sync.dma_start(out=slice0[ds(i * P, P), :], in_=xf[:])
            nc.gpsimd.collective_compute(
                "AllGather", mybir.AluOpType.bypass, replica_groups=groups,
                ins=[slice0[:]], outs=[tab1[:]])

            buf_lo = gbufp.tile([P, NT_lo * P], F32, tag="buflo")
            buf_hi = gbufp.tile([P, NT_hi * P], F32, tag="bufhi")
            pool_ps = {}
            for half in ("A", "B"):
                t = ppool.tile([P, F], F32, space="PSUM",
                               tag=f"pool{half}", name=f"pool{half}")
                nc.vector.memset(t[:], 0.0)
                pool_ps[half] = t

            def emit_layer(Lno, tab, W_sb, bias_col, dst_scale, out_slice):
                qi = 0
                with tc.For_i(0, NBLK, 1, name=f"layer{Lno}") as i:
                    streams = [
                        ("lo", NT_lo, idxlo, buf_lo, tab[0:NHALF, :], 0),
                        ("hi", NT_hi, idxhi, buf_hi, tab[NHALF:NPAD, :], NT_lo),
                    ]
                    for sname, NT, idx_sb, buf, tab_ap, coff in streams:
                        for (t0, nt) in chunks(NT):
                            nc.gpsimd.dma_gather(
                                out_ap=buf[:, t0 * P:(t0 + nt) * P].rearrange(
                                    "p (c f) -> p c f", f=F),
                                in_ap=tab_ap,
                                idxs_ap=idx_sb[:, ds(i * NT * 8 + t0 * 8, nt * 8)],
                                num_idxs=nt * P, num_idxs_reg=nt * P,
                                elem_size=F, single_packet=True,
                                queue_num=qi % NQ)
                            qi += 1
                    agg_ps = pagg.tile([P, F], F32, space="PSUM", tag="agg")
                    k = 0
                    for sname, NT, idx_sb, buf, tab_ap, coff in streams:
                        for t in range(NT):
                            sel = selp.tile([P, P], F32)
                            col = ds(i * NTT + coff + t, 1)
                            nc.vector.tensor_scalar(
                                out=sel[:], in0=iota[:],
                                scalar1=doff[:, col], scalar2=None,
                                op0=mybir.AluOpType.is_equal)
                            if dst_scale:
                                # node-major: agg[d, f] += sel^T msg
                                nc.tensor.matmul(
                                    out=agg_ps[:], lhsT=sel[:],
                                    rhs=buf[:, t * P:(t + 1) * P],
                                    start=(k == 0), stop=(k == NTT - 1))
                            else:
                                # feat-major: aggT[f, d] += msg^T sel
                                nc.tensor.matmul(
                                    out=agg_ps[:],
                                    lhsT=buf[:, t * P:(t + 1) * P],
                                    rhs=sel[:], start=(k == 0),
                                    stop=(k == NTT - 1))
                            k += 1
                    if dst_scale:
                        # scale rows (dst nodes) by dinv[d], then transpose
                        # to feat-major for the W transform
                        agg_sb = workp.tile([P, F], F32, tag="aggnm")
                        nc.vector.tensor_scalar(
                            out=agg_sb[:], in0=agg_ps[:],
                            scalar1=dinvb[:, ds(i, 1)], scalar2=None,
                            op0=mybir.AluOpType.mult)
                        aggT_ps = ptr.tile([P, F], F32, space="PSUM", tag="tr0")
                        nc.tensor.transpose(out=aggT_ps[:], in_=agg_sb[:],
                                            identity=ident[:])
                        aggT = workp.tile([P, F], F32, tag="aggT")
                        nc.vector.tensor_copy(out=aggT[:], in_=aggT_ps[:])
                    else:
                        aggT = workp.tile([P, F], F32, tag="aggT")
                        nc.vector.tensor_copy(out=aggT[:], in_=agg_ps[:])
                    hT_ps = phT.tile([P, F], F32, space="PSUM", tag="hT")
                    nc.tensor.matmul(out=hT_ps[:], lhsT=W_sb[:], rhs=aggT[:],
                                     start=True, stop=True)
                    hT = workp.tile([P, F], F32, tag="hT_sb")
                    nc.scalar.activation(out=hT[:], in_=hT_ps[:],
                                         func=mybir.ActivationFunctionType.Relu,
                                         bias=bias_col[:, 0:1])
                    h_ps = ptr.tile([P, F], F32, space="PSUM", tag="tr1")
                    nc.tensor.transpose(out=h_ps[:], in_=hT[:], identity=ident[:])
                    h_sb = workp.tile([P, F], F32, tag="h_sb")
                    nc.vector.tensor_copy(out=h_sb[:], in_=h_ps[:])
                    if out_slice is not None:
                        nc.sync.dma_start(out=out_slice[ds(i * P, P), :],
                                          in_=h_sb[:])
                    else:
                        for hi, half in enumerate(("A", "B")):
                            selp_t = selp.tile([P, P], F32)
                            nc.vector.tensor_scalar(
                                out=selp_t[:], in0=iota[:],
                                scalar1=batf[:, ds(hi * NBLK + i, 1)],
                                scalar2=None,
                                op0=mybir.AluOpType.is_equal)
                            nc.tensor.matmul(
                                out=pool_ps[half][:], lhsT=selp_t[:],
                                rhs=h_sb[:], start=False, stop=False)

            emit_layer(0, tab1, w0, b0c, True, slice1)
            nc.gpsimd.collective_compute(
                "AllGather", mybir.AluOpType.bypass, replica_groups=groups,
                ins=[slice1[:]], outs=[tab2[:]])
            emit_layer(1, tab2, wg1, bg1c, False, slice2)
            nc.gpsimd.collective_compute(
                "AllGather", mybir.AluOpType.bypass, replica_groups=groups,
                ins=[slice2[:]], outs=[tab3[:]])
            emit_layer(2, tab3, wg2, bg2c, False, None)

            # ---- pooling: close PSUM groups, partial means -> AllReduce
            for hi, half in enumerate(("A", "B")):
                nc.tensor.matmul(out=pool_ps[half][:], lhsT=zero_sb[:],
                                 rhs=zero_sb[:], start=False, stop=True)
                m_sb = workp.tile([P, F], F32, tag=f"m{half}")
                nc.vector.tensor_scalar(
                    out=m_sb[:], in0=pool_ps[half][:],
                    scalar1=invcf[:, hi:hi + 1],
                    scalar2=None, op0=mybir.AluOpType.mult)
                nc.sync.dma_start(out=pool_in[hi * P:(hi + 1) * P, :], in_=m_sb[:])
            nc.gpsimd.collective_compute(
                "AllReduce", mybir.AluOpType.add, replica_groups=groups,
                ins=[pool_in[:]], outs=[pool_out[:]])

            # ---- head (redundant on every core)
            g1T = {}
            for hi, half in enumerate(("A", "B")):
                m_sb = workp.tile([P, F], F32, tag=f"mf{half}")
                nc.sync.dma_start(out=m_sb[:], in_=pool_out[hi * P:(hi + 1) * P, :])
                mT_ps = phT.tile([P, F], F32, space="PSUM", tag="hT")
                nc.tensor.transpose(out=mT_ps[:], in_=m_sb[:], identity=ident[:])
                mT = workp.tile([P, F], F32, tag=f"mT{half}")
                nc.vector.tensor_copy(out=mT[:], in_=mT_ps[:])
                for h in range(NHID // P):
                    g_ps = pagg.tile([P, P], F32, space="PSUM", tag="agg")
                    nc.tensor.matmul(out=g_ps[:], lhsT=wh1[:, h * P:(h + 1) * P],
                                     rhs=mT[:], start=True, stop=True)
                    gt = workp.tile([P, P], F32, tag=f"g1T{half}{h}")
                    nc.scalar.activation(out=gt[:], in_=g_ps[:],
                                         func=mybir.ActivationFunctionType.Relu,
                                         bias=bh1c[:, h:h + 1])
                    g1T[(half, h)] = gt
            for hi, half in enumerate(("A", "B")):
                o_ps = pagg.tile([P, NOUT], F32, space="PSUM", tag="agg")
                for h in range(NHID // P):
                    nc.tensor.matmul(out=o_ps[:], lhsT=g1T[(half, h)][:],
                                     rhs=wh2[:, h * NOUT:(h + 1) * NOUT],
                                     start=(h == 0), stop=False)
                nc.tensor.matmul(out=o_ps[:], lhsT=ones1[:], rhs=bh2row[:],
                                 start=False, stop=True)
                o_sb = workp.tile([P, NOUT], F32, tag=f"o{half}")
                nc.vector.tensor_copy(out=o_sb[:], in_=o_ps[:])
                nc.sync.dma_start(out=out_d[hi * P:(hi + 1) * P, :], in_=o_sb[:])

    nc.compile()
    return nc


_CACHE = {}


def run(cfg, inputs):
    in_maps, meta = preprocess(cfg, **inputs)
    key = (cfg.N, meta["NT_lo"], meta["NT_hi"])
    if key not in _CACHE:
        _CACHE[key] = build_program(cfg, meta)
    nc = _CACHE[key]
    res = run_bass_kernel_spmd(nc, in_maps, core_ids=list(range(NCORES)))
    return res.results[0]["out"].astype(np.float32)


def kernel(**inputs):
    return run(FULL, inputs)
